# revision 19
# baseline (speedup 1.0000x reference)
"""Data-parallel FFLayer kernel for 8 TRN2 NeuronCores (Bass/Tile).

Computes  out = relu( (x / (||x||_2_row + 1e-4)) @ W.T + b )  for
x [16384, 2048], W [2048, 2048], b [2048], all float32.

Sharding (data-parallel): x is split along batch into 8 shards of
[2048, 2048]; W and b are replicated.  Host-side staging is layout
permutation + the bf16 rounding the device matmul performs anyway:
  * W shipped as W.T in [P, NK, OUT] bf16 (partition-major so ko
    ranges batch into single DMAs with matching element order).
  * x shipped twice in bf16: blocked-transpose lhsT (a k-interleaved
    block for bt0-2 so the startup stream delivers lhsT k-slices just
    ahead of the W k-stream, flat for bt3+) and row-major xbf (norm).
  * output written bf16 and upcast on host (rel-err budget 2e-2).

Per-core schedule (PE roofline 218.5us = 1024 matmuls x 512 rows at
2.4 GHz; input stream before bt2 can start = ~10.9 MB = ~31us, which
is the startup bound):
  1. Startup: bt0 and bt1 interleave ko-by-ko (2 x 0.86us of PE work
     per ~1.55us W-tile arrival) so the PE tracks the W stream without
     idling.  Uses all 8 PSUM banks (2 b-tiles x 4 chunk banks).
  2. One serial input DMA stream on the sync queue, priority-ordered;
     each dma_start costs ~0.4us so batches are as large as arrival
     granularity allows.  Out DMAs + the xbf ring-wrap ride the
     scalar queue (their data deps hold them back; a dep-free DMA on
     a second queue would fire immediately and steal fabric bandwidth
     from the W stream -- measured 10us loss).
  3. Norm chain: ACT Square+accum -> Sqrt; the two small DVE ops
     (+eps, reciprocal) are emitted immediately before the consuming
     evict's STTs so they can never delay a PSUM release.
  4. Evict: DVE scalar_tensor_tensor (psum*s + bias) then ACT Relu
     into a bf16 out tile.
  5. Last bt runs chunk-major so chunks complete staggered by ~3.4us;
     per-chunk evict+DMA shortens the kernel tail to ~3us.
"""

import numpy as np

B, IN, OUT, NCORES = 16384, 2048, 2048, 8
BS = B // NCORES  # batch rows per core
P = 128
NB = BS // P  # b-tiles per core
NK = IN // P  # k-tiles
XBF_SLOTS = 8  # xbf ring slots in SBUF
NI = 3  # b-tiles in the k-interleaved startup block (bt0..2)
NBT = NB - NI  # b-tiles in the flat xt (bt3..15)

_NC_CACHE = {}


def _build_nc():
    import concourse.mybir as mybir
    import concourse.tile as tile
    from concourse import bacc

    f32 = mybir.dt.float32
    bf16 = mybir.dt.bfloat16
    AF = mybir.ActivationFunctionType
    ALU = mybir.AluOpType

    nc = bacc.Bacc()
    # xti[ki, ko, j, b]: k-interleaved lhsT for bt j in 0..NI-1
    xti_d = nc.declare_dram_parameter("xti", [P, NK * NI * P], bf16, isOutput=False)
    # xt[ki, (bt-NI, ko, b)]: flat lhsT for bt NI..15
    xt_d = nc.declare_dram_parameter("xt", [P, NBT * NK * P], bf16, isOutput=False)
    xbf_d = nc.declare_dram_parameter("xbf", [P, NB, IN], bf16, isOutput=False)
    wt_d = nc.declare_dram_parameter("wt", [P, NK, OUT], bf16, isOutput=False)
    b_d = nc.declare_dram_parameter("bias", [P, OUT], f32, isOutput=False)
    out_d = nc.declare_dram_parameter("out", [BS, OUT], bf16, isOutput=True)

    with tile.TileContext(nc) as tc:
        with (
            tc.tile_pool(name="consts", bufs=1) as consts,
            tc.tile_pool(name="sq", bufs=2) as sqp,
            tc.tile_pool(name="outp", bufs=3) as outp,
            tc.tile_pool(name="small", bufs=24) as small,
            tc.tile_pool(name="po", bufs=8, space="PSUM") as pop,
        ):
            wt_all = consts.tile([P, NK, OUT], bf16)
            xti_sb = consts.tile([P, NK * NI * P], bf16)
            xt_all = consts.tile([P, NBT * NK * P], bf16)
            xbf_sb = consts.tile([P, XBF_SLOTS, IN], bf16)
            bias_sb = consts.tile([P, OUT], f32)

            # Warm the Square/Sqrt ACT table (one set: sqrt_and_others
            # covers square+sqrt+relu+copy) while DMA streams in.
            warm = consts.tile([P, 1], f32)
            nc.vector.memset(warm, 1.0)
            nc.scalar.activation(out=warm, in_=warm, func=AF.Square)
            nc.scalar.activation(out=warm, in_=warm, func=AF.Sqrt)

            # --- input DMA stream (sync queue), priority order -------
            # W rides as single k-tiles from wt2 on: the startup
            # interleave consumes ~1.72us per tile vs ~1.55us arrival,
            # so finer granularity avoids phase-mismatch stalls.  The
            # first-evict inputs (xbf0/1, bias halves) are interleaved
            # into the last W slots so s0 and bias are ready right as
            # the startup interleave drains (~42us).
            KPI = NI * P  # one ko of the interleaved block
            nc.sync.dma_start(xti_sb[:, 0 : 2 * KPI], xti_d[:, 0 : 2 * KPI])
            nc.sync.dma_start(wt_all[:, 0], wt_d[:, 0])
            nc.sync.dma_start(xti_sb[:, 2 * KPI : 8 * KPI], xti_d[:, 2 * KPI : 8 * KPI])
            nc.sync.dma_start(wt_all[:, 1], wt_d[:, 1])
            nc.sync.dma_start(
                xti_sb[:, 8 * KPI : 16 * KPI], xti_d[:, 8 * KPI : 16 * KPI]
            )
            for k in range(1, 8):
                nc.sync.dma_start(
                    wt_all[:, 2 * k : 2 * k + 2], wt_d[:, 2 * k : 2 * k + 2]
                )
            nc.sync.dma_start(xbf_sb[:, 0:1], xbf_d[:, 0:1])
            nc.sync.dma_start(bias_sb[:, 0:1024], b_d[:, 0:1024])
            nc.sync.dma_start(xbf_sb[:, 1:2], xbf_d[:, 1:2])
            nc.sync.dma_start(bias_sb[:, 1024:2048], b_d[:, 1024:2048])
            BTW = NK * P  # one bt's worth of xt columns
            nc.sync.dma_start(xt_all[:, 0 : 2 * BTW], xt_d[:, 0 : 2 * BTW])
            nc.sync.dma_start(xbf_sb[:, 2:8], xbf_d[:, 2:8])
            nc.sync.dma_start(xt_all[:, 2 * BTW :], xt_d[:, 2 * BTW :])

            def norm_act(bt):
                """ACT half of the norm: square + row-accum, sqrt."""
                sq = sqp.tile([P, IN], bf16, tag="sq")
                nsq = small.tile([P, 1], f32, tag="nsq")
                nc.scalar.activation(
                    out=sq,
                    in_=xbf_sb[:, bt % XBF_SLOTS],
                    func=AF.Square,
                    accum_out=nsq,
                )
                nrm = small.tile([P, 1], f32, tag="nrm")
                nc.scalar.activation(out=nrm, in_=nsq, func=AF.Sqrt)
                return nrm

            def norm_dve(nrm):
                """DVE half: s = 1/(nrm+eps).  Emitted right before the
                consuming evict so the in-order DVE queue never holds a
                PSUM-freeing STT behind a late norm."""
                nrm2 = small.tile([P, 1], f32, tag="nrm2")
                nc.vector.tensor_scalar_add(nrm2, nrm, 1e-4)
                s = small.tile([P, 1], f32, tag="s")
                nc.vector.reciprocal(s, nrm2)
                return s

            def lhsT(bt, ko):
                if bt < NI:
                    return xti_sb[:, (ko * NI + bt) * P : (ko * NI + bt + 1) * P]
                i = ((bt - NI) * NK + ko) * P
                return xt_all[:, i : i + P]

            def alloc_ps():
                return [
                    pop.tile([P, 512], f32, tag="ps", name=f"ps{c}")
                    for c in range(4)
                ]

            def mm(bt, ko, ps, c):
                nc.tensor.matmul(
                    ps[c],
                    lhsT=lhsT(bt, ko),
                    rhs=wt_all[:, ko, c * 512 : (c + 1) * 512],
                    start=(ko == 0),
                    stop=(ko == NK - 1),
                )

            def evict(bt, ps, nrm):
                # out = relu(ps * s[b] + bias[o]); STT on DVE frees the
                # PSUM bank, Relu on ACT (merged 1024-col halves: fewer
                # ACT instructions -> shorter end-of-kernel drain),
                # bf16 out DMA issued from the idle sync queue.
                s = norm_dve(nrm)
                o_sb = outp.tile([P, OUT], bf16, tag="o_sb")
                for c in range(4):
                    lo = c * 512
                    nc.vector.scalar_tensor_tensor(
                        o_sb[:, lo : lo + 512],
                        ps[c],
                        s,
                        bias_sb[:, lo : lo + 512],
                        ALU.mult,
                        ALU.add,
                    )
                    if c % 2 == 1:
                        nc.scalar.activation(
                            o_sb[:, lo - 512 : lo + 512],
                            o_sb[:, lo - 512 : lo + 512],
                            AF.Relu,
                        )
                nc.sync.dma_start(out_d[bt * P : (bt + 1) * P, :], o_sb)

            # --- schedule -------------------------------------------
            nrms = {0: norm_act(0), 1: norm_act(1)}
            ps_of = {0: alloc_ps(), 1: alloc_ps()}
            # Warm-up dummies: ~5us of garbage matmuls on a memset tile
            # keep the PE continuously busy from the preamble until the
            # first real operands land, so the DVFS ramp (0.65 -> 1.2
            # -> 2.4 GHz after 3us continuous) completes beforehand and
            # every real matmul runs at full clock.  They write into
            # bt0's PSUM tile; the real ko0 start=True overwrites.
            zt = consts.tile([P, 512], bf16)
            nc.vector.memset(zt, 0.0)
            for _ in range(9):
                nc.tensor.matmul(
                    ps_of[0][0], lhsT=zt[:, 0:P], rhs=zt, start=True, stop=True
                )
            # Startup: interleave bt0/bt1 per ko to track W arrival.
            for ko in range(NK):
                for c in range(4):
                    mm(0, ko, ps_of[0], c)
                for c in range(4):
                    mm(1, ko, ps_of[1], c)
            evict(0, ps_of[0], nrms.pop(0))
            evict(1, ps_of[1], nrms.pop(1))
            del ps_of[0], ps_of[1]
            for bt in (2, 3, 4, 5, 6, 7):
                nrms[bt] = norm_act(bt)
            # Ring wrap (slots 0..7 -> bt 8..15) after the squares of
            # bt 0..7 in program order; its WAR deps on those squares
            # hold the transfer back, so the scalar queue is safe.
            nc.scalar.dma_start(xbf_sb[:, 0:8], xbf_d[:, 8:16])

            for bt in range(2, NB - 1):
                ps = alloc_ps()
                for ko in range(NK):
                    for c in range(4):
                        mm(bt, ko, ps, c)
                evict(bt, ps, nrms.pop(bt))
                if bt + 6 < NB:
                    nrms[bt + 6] = norm_act(bt + 6)

            # Last bt chunk-major: chunks complete staggered by ~3.4us,
            # so all but the last chunk's evict hides under matmuls.
            bt = NB - 1
            ps = alloc_ps()
            s_last = norm_dve(nrms.pop(bt))
            o_sb = outp.tile([P, OUT], bf16, tag="o_sb")
            # Chunk-major with per-chunk evict+DMA: chunks complete
            # staggered, only the last chunk's chain is exposed.
            pieces = [(0, 512), (512, 512), (1024, 512), (1536, 512)]
            for lo, w in pieces:
                pt = ps[lo // 512][:, lo % 512 : lo % 512 + w]
                for ko in range(NK):
                    nc.tensor.matmul(
                        pt,
                        lhsT=lhsT(bt, ko),
                        rhs=wt_all[:, ko, lo : lo + w],
                        start=(ko == 0),
                        stop=(ko == NK - 1),
                    )
                nc.vector.scalar_tensor_tensor(
                    o_sb[:, lo : lo + w],
                    pt,
                    s_last,
                    bias_sb[:, lo : lo + w],
                    ALU.mult,
                    ALU.add,
                )
                nc.scalar.activation(
                    o_sb[:, lo : lo + w], o_sb[:, lo : lo + w], AF.Relu
                )
                nc.sync.dma_start(
                    out_d[bt * P : (bt + 1) * P, lo : lo + w],
                    o_sb[:, lo : lo + w],
                )

    nc.compile()
    return nc


def _get_nc():
    if "nc" not in _NC_CACHE:
        _NC_CACHE["nc"] = _build_nc()
    return _NC_CACHE["nc"]


def _make_in_maps(x, W, b):
    import ml_dtypes

    x = np.ascontiguousarray(np.asarray(x, dtype=np.float32))
    W = np.asarray(W, dtype=np.float32)
    b = np.asarray(b, dtype=np.float32)
    # host-side staging: layout permutations + the bf16 rounding the
    # device matmul performs anyway
    wt = np.ascontiguousarray(
        W.T.astype(ml_dtypes.bfloat16).reshape(NK, P, OUT).transpose(1, 0, 2)
    )
    bias = np.ascontiguousarray(np.broadcast_to(b.reshape(1, OUT), (P, OUT)))
    in_maps = []
    for i in range(NCORES):
        xs = x[i * BS : (i + 1) * BS].astype(ml_dtypes.bfloat16)
        xq = xs.reshape(NB, P, NK, P)  # [bt, b, ko, ki]
        # xti[ki, ko, j, b]: k-interleaved bt0..2 for the startup
        xti = np.ascontiguousarray(
            xq[0:NI].transpose(3, 2, 0, 1).reshape(P, NK * NI * P)
        )
        # xt[ki, bt-NI, ko, b] flat for bt3..15
        xt = np.ascontiguousarray(
            xq[NI:].transpose(3, 0, 2, 1).reshape(P, NBT * NK * P)
        )
        # xbf[b, bt, i] = x[bt*128+b, i]  (norm input)
        xbf = np.ascontiguousarray(xs.reshape(NB, P, IN).transpose(1, 0, 2))
        in_maps.append(
            {"xti": xti, "xt": xt, "xbf": xbf, "wt": wt, "bias": bias}
        )
    return in_maps


def _run(x, W, b, trace=False):
    from concourse.bass_utils import run_bass_kernel_spmd

    nc = _get_nc()
    res = run_bass_kernel_spmd(
        nc, _make_in_maps(x, W, b), core_ids=list(range(NCORES)), trace=trace
    )
    out = np.concatenate(
        [
            np.asarray(res.results[i]["out"]).astype(np.float32)
            for i in range(NCORES)
        ],
        axis=0,
    )
    return out, res


def kernel(**inputs):
    out, _ = _run(inputs["x"], inputs["W"], inputs["b"])
    return out


def run_profiled(**inputs):
    out, res = _run(inputs["x"], inputs["W"], inputs["b"], trace=True)
    return out, res


# revision 21
# speedup vs baseline: 1.0108x; 1.0108x over previous
"""Data-parallel FFLayer kernel for 8 TRN2 NeuronCores (Bass/Tile).

Computes  out = relu( (x / (||x||_2_row + 1e-4)) @ W.T + b )  for
x [16384, 2048], W [2048, 2048], b [2048], all float32.

Sharding (data-parallel): x is split along batch into 8 shards of
[2048, 2048]; W and b are replicated.  Host-side staging is layout
permutation + the bf16 rounding the device matmul performs anyway:
  * W shipped as W.T in [P, NK, OUT] bf16 (partition-major so ko
    ranges batch into single DMAs with matching element order).
  * x shipped twice in bf16: blocked-transpose lhsT (a k-interleaved
    block for bt0-2 so the startup stream delivers lhsT k-slices just
    ahead of the W k-stream, flat for bt3+) and row-major xbf (norm).
  * output written bf16 and upcast on host (rel-err budget 2e-2).

Per-core schedule (PE roofline 218.5us = 1024 matmuls x 512 rows at
2.4 GHz; input stream before bt2 can start = ~10.9 MB = ~31us, which
is the startup bound):
  1. Startup: bt0 and bt1 interleave ko-by-ko (2 x 0.86us of PE work
     per ~1.55us W-tile arrival) so the PE tracks the W stream without
     idling.  Uses all 8 PSUM banks (2 b-tiles x 4 chunk banks).
  2. One serial input DMA stream on the sync queue, priority-ordered;
     each dma_start costs ~0.4us so batches are as large as arrival
     granularity allows.  Out DMAs + the xbf ring-wrap ride the
     scalar queue (their data deps hold them back; a dep-free DMA on
     a second queue would fire immediately and steal fabric bandwidth
     from the W stream -- measured 10us loss).
  3. Norm chain: ACT Square+accum -> Sqrt; the two small DVE ops
     (+eps, reciprocal) are emitted immediately before the consuming
     evict's STTs so they can never delay a PSUM release.
  4. Evict: DVE scalar_tensor_tensor (psum*s + bias) then ACT Relu
     into a bf16 out tile.
  5. Last bt runs chunk-major so chunks complete staggered by ~3.4us;
     per-chunk evict+DMA shortens the kernel tail to ~3us.
"""

import numpy as np

B, IN, OUT, NCORES = 16384, 2048, 2048, 8
BS = B // NCORES  # batch rows per core
P = 128
NB = BS // P  # b-tiles per core
NK = IN // P  # k-tiles
XBF_SLOTS = 8  # xbf ring slots in SBUF
NI = 3  # b-tiles in the k-interleaved startup block (bt0..2)
NBT = NB - NI  # b-tiles in the flat xt (bt3..15)

_NC_CACHE = {}


def _build_nc():
    import concourse.mybir as mybir
    import concourse.tile as tile
    from concourse import bacc

    f32 = mybir.dt.float32
    bf16 = mybir.dt.bfloat16
    AF = mybir.ActivationFunctionType
    ALU = mybir.AluOpType

    nc = bacc.Bacc()
    # xti[ki, ko, j, b]: k-interleaved lhsT for bt j in 0..NI-1
    xti_d = nc.declare_dram_parameter("xti", [P, NK * NI * P], bf16, isOutput=False)
    # xt[ki, (bt-NI, ko, b)]: flat lhsT for bt NI..15
    xt_d = nc.declare_dram_parameter("xt", [P, NBT * NK * P], bf16, isOutput=False)
    xbf_d = nc.declare_dram_parameter("xbf", [P, NB, IN], bf16, isOutput=False)
    wt_d = nc.declare_dram_parameter("wt", [P, NK, OUT], bf16, isOutput=False)
    b_d = nc.declare_dram_parameter("bias", [P, OUT], f32, isOutput=False)
    out_d = nc.declare_dram_parameter("out", [BS, OUT], bf16, isOutput=True)

    with tile.TileContext(nc) as tc:
        with (
            tc.tile_pool(name="consts", bufs=1) as consts,
            tc.tile_pool(name="sq", bufs=2) as sqp,
            tc.tile_pool(name="outp", bufs=3) as outp,
            tc.tile_pool(name="small", bufs=24) as small,
            tc.tile_pool(name="po", bufs=8, space="PSUM") as pop,
        ):
            wt_all = consts.tile([P, NK, OUT], bf16)
            xti_sb = consts.tile([P, NK * NI * P], bf16)
            xt_all = consts.tile([P, NBT * NK * P], bf16)
            xbf_sb = consts.tile([P, XBF_SLOTS, IN], bf16)
            bias_sb = consts.tile([P, OUT], f32)

            # Warm the Square/Sqrt ACT table (one set: sqrt_and_others
            # covers square+sqrt+relu+copy) while DMA streams in.
            warm = consts.tile([P, 1], f32)
            nc.vector.memset(warm, 1.0)
            nc.scalar.activation(out=warm, in_=warm, func=AF.Square)
            nc.scalar.activation(out=warm, in_=warm, func=AF.Sqrt)

            # --- input DMA stream (sync queue), priority order -------
            # W rides as single k-tiles from wt2 on: the startup
            # interleave consumes ~1.72us per tile vs ~1.55us arrival,
            # so finer granularity avoids phase-mismatch stalls.  The
            # first-evict inputs (xbf0/1, bias halves) are interleaved
            # into the last W slots so s0 and bias are ready right as
            # the startup interleave drains (~42us).
            KPI = NI * P  # one ko of the interleaved block
            nc.sync.dma_start(xti_sb[:, 0 : 2 * KPI], xti_d[:, 0 : 2 * KPI])
            nc.sync.dma_start(wt_all[:, 0], wt_d[:, 0])
            nc.sync.dma_start(xti_sb[:, 2 * KPI : 8 * KPI], xti_d[:, 2 * KPI : 8 * KPI])
            nc.sync.dma_start(wt_all[:, 1], wt_d[:, 1])
            nc.sync.dma_start(
                xti_sb[:, 8 * KPI : 16 * KPI], xti_d[:, 8 * KPI : 16 * KPI]
            )
            for k in range(2, 12):
                nc.sync.dma_start(wt_all[:, k], wt_d[:, k])
            nc.sync.dma_start(xbf_sb[:, 0:1], xbf_d[:, 0:1])
            nc.sync.dma_start(wt_all[:, 12], wt_d[:, 12])
            nc.sync.dma_start(bias_sb[:, 0:1024], b_d[:, 0:1024])
            nc.sync.dma_start(wt_all[:, 13], wt_d[:, 13])
            nc.sync.dma_start(xbf_sb[:, 1:2], xbf_d[:, 1:2])
            nc.sync.dma_start(wt_all[:, 14], wt_d[:, 14])
            nc.sync.dma_start(wt_all[:, 15], wt_d[:, 15])
            nc.sync.dma_start(bias_sb[:, 1024:2048], b_d[:, 1024:2048])
            BTW = NK * P  # one bt's worth of xt columns
            nc.sync.dma_start(xt_all[:, 0 : 2 * BTW], xt_d[:, 0 : 2 * BTW])
            nc.sync.dma_start(xbf_sb[:, 2:8], xbf_d[:, 2:8])
            nc.sync.dma_start(xt_all[:, 2 * BTW :], xt_d[:, 2 * BTW :])

            def norm_act(bt):
                """ACT half of the norm: square + row-accum, sqrt."""
                sq = sqp.tile([P, IN], bf16, tag="sq")
                nsq = small.tile([P, 1], f32, tag="nsq")
                nc.scalar.activation(
                    out=sq,
                    in_=xbf_sb[:, bt % XBF_SLOTS],
                    func=AF.Square,
                    accum_out=nsq,
                )
                nrm = small.tile([P, 1], f32, tag="nrm")
                nc.scalar.activation(out=nrm, in_=nsq, func=AF.Sqrt)
                return nrm

            def norm_dve(nrm):
                """DVE half: s = 1/(nrm+eps).  Emitted right before the
                consuming evict so the in-order DVE queue never holds a
                PSUM-freeing STT behind a late norm."""
                nrm2 = small.tile([P, 1], f32, tag="nrm2")
                nc.vector.tensor_scalar_add(nrm2, nrm, 1e-4)
                s = small.tile([P, 1], f32, tag="s")
                nc.vector.reciprocal(s, nrm2)
                return s

            def lhsT(bt, ko):
                if bt < NI:
                    return xti_sb[:, (ko * NI + bt) * P : (ko * NI + bt + 1) * P]
                i = ((bt - NI) * NK + ko) * P
                return xt_all[:, i : i + P]

            def alloc_ps():
                return [
                    pop.tile([P, 512], f32, tag="ps", name=f"ps{c}")
                    for c in range(4)
                ]

            def mm(bt, ko, ps, c):
                nc.tensor.matmul(
                    ps[c],
                    lhsT=lhsT(bt, ko),
                    rhs=wt_all[:, ko, c * 512 : (c + 1) * 512],
                    start=(ko == 0),
                    stop=(ko == NK - 1),
                )

            def evict(bt, ps, nrm):
                # out = relu(ps * s[b] + bias[o]); STT on DVE frees the
                # PSUM bank, Relu on ACT (merged 1024-col halves: fewer
                # ACT instructions -> shorter end-of-kernel drain),
                # bf16 out DMA issued from the idle sync queue.
                s = norm_dve(nrm)
                o_sb = outp.tile([P, OUT], bf16, tag="o_sb")
                for c in range(4):
                    lo = c * 512
                    nc.vector.scalar_tensor_tensor(
                        o_sb[:, lo : lo + 512],
                        ps[c],
                        s,
                        bias_sb[:, lo : lo + 512],
                        ALU.mult,
                        ALU.add,
                    )
                    if c % 2 == 1:
                        nc.scalar.activation(
                            o_sb[:, lo - 512 : lo + 512],
                            o_sb[:, lo - 512 : lo + 512],
                            AF.Relu,
                        )
                nc.sync.dma_start(out_d[bt * P : (bt + 1) * P, :], o_sb)

            # --- schedule -------------------------------------------
            nrms = {0: norm_act(0), 1: norm_act(1)}
            ps_of = {0: alloc_ps(), 1: alloc_ps()}
            # Warm-up dummies: ~5us of garbage matmuls on a memset tile
            # keep the PE continuously busy from the preamble until the
            # first real operands land, so the DVFS ramp (0.65 -> 1.2
            # -> 2.4 GHz after 3us continuous) completes beforehand and
            # every real matmul runs at full clock.  They write into
            # bt0's PSUM tile; the real ko0 start=True overwrites.
            zt = consts.tile([P, 512], bf16)
            nc.vector.memset(zt, 0.0)
            for _ in range(9):
                nc.tensor.matmul(
                    ps_of[0][0], lhsT=zt[:, 0:P], rhs=zt, start=True, stop=True
                )
            # Startup: interleave bt0/bt1 per ko to track W arrival.
            for ko in range(NK):
                for c in range(4):
                    mm(0, ko, ps_of[0], c)
                for c in range(4):
                    mm(1, ko, ps_of[1], c)
            evict(0, ps_of[0], nrms.pop(0))
            evict(1, ps_of[1], nrms.pop(1))
            del ps_of[0], ps_of[1]
            for bt in (2, 3, 4, 5, 6, 7):
                nrms[bt] = norm_act(bt)
            # Ring wrap (slots 0..7 -> bt 8..15) after the squares of
            # bt 0..7 in program order; its WAR deps on those squares
            # hold the transfer back, so the scalar queue is safe.
            nc.scalar.dma_start(xbf_sb[:, 0:8], xbf_d[:, 8:16])

            for bt in range(2, NB - 1):
                ps = alloc_ps()
                for ko in range(NK):
                    for c in range(4):
                        mm(bt, ko, ps, c)
                evict(bt, ps, nrms.pop(bt))
                if bt + 6 < NB:
                    nrms[bt + 6] = norm_act(bt + 6)

            # Last bt chunk-major: chunks complete staggered by ~3.4us,
            # so all but the last chunk's evict hides under matmuls.
            bt = NB - 1
            ps = alloc_ps()
            s_last = norm_dve(nrms.pop(bt))
            o_sb = outp.tile([P, OUT], bf16, tag="o_sb")
            # The final chunk is split in two 256-col pieces to halve
            # the very last evict chain (STT+Relu+DMA).
            pieces = [(0, 512), (512, 512), (1024, 512), (1536, 256), (1792, 256)]
            for lo, w in pieces:
                pt = ps[lo // 512][:, lo % 512 : lo % 512 + w]
                for ko in range(NK):
                    nc.tensor.matmul(
                        pt,
                        lhsT=lhsT(bt, ko),
                        rhs=wt_all[:, ko, lo : lo + w],
                        start=(ko == 0),
                        stop=(ko == NK - 1),
                    )
                nc.vector.scalar_tensor_tensor(
                    o_sb[:, lo : lo + w],
                    pt,
                    s_last,
                    bias_sb[:, lo : lo + w],
                    ALU.mult,
                    ALU.add,
                )
                nc.scalar.activation(
                    o_sb[:, lo : lo + w], o_sb[:, lo : lo + w], AF.Relu
                )
                nc.sync.dma_start(
                    out_d[bt * P : (bt + 1) * P, lo : lo + w],
                    o_sb[:, lo : lo + w],
                )

    nc.compile()
    return nc


def _get_nc():
    if "nc" not in _NC_CACHE:
        _NC_CACHE["nc"] = _build_nc()
    return _NC_CACHE["nc"]


def _make_in_maps(x, W, b):
    import ml_dtypes

    x = np.ascontiguousarray(np.asarray(x, dtype=np.float32))
    W = np.asarray(W, dtype=np.float32)
    b = np.asarray(b, dtype=np.float32)
    # host-side staging: layout permutations + the bf16 rounding the
    # device matmul performs anyway
    wt = np.ascontiguousarray(
        W.T.astype(ml_dtypes.bfloat16).reshape(NK, P, OUT).transpose(1, 0, 2)
    )
    bias = np.ascontiguousarray(np.broadcast_to(b.reshape(1, OUT), (P, OUT)))
    in_maps = []
    for i in range(NCORES):
        xs = x[i * BS : (i + 1) * BS].astype(ml_dtypes.bfloat16)
        xq = xs.reshape(NB, P, NK, P)  # [bt, b, ko, ki]
        # xti[ki, ko, j, b]: k-interleaved bt0..2 for the startup
        xti = np.ascontiguousarray(
            xq[0:NI].transpose(3, 2, 0, 1).reshape(P, NK * NI * P)
        )
        # xt[ki, bt-NI, ko, b] flat for bt3..15
        xt = np.ascontiguousarray(
            xq[NI:].transpose(3, 0, 2, 1).reshape(P, NBT * NK * P)
        )
        # xbf[b, bt, i] = x[bt*128+b, i]  (norm input)
        xbf = np.ascontiguousarray(xs.reshape(NB, P, IN).transpose(1, 0, 2))
        in_maps.append(
            {"xti": xti, "xt": xt, "xbf": xbf, "wt": wt, "bias": bias}
        )
    return in_maps


def _run(x, W, b, trace=False):
    from concourse.bass_utils import run_bass_kernel_spmd

    nc = _get_nc()
    res = run_bass_kernel_spmd(
        nc, _make_in_maps(x, W, b), core_ids=list(range(NCORES)), trace=trace
    )
    out = np.concatenate(
        [
            np.asarray(res.results[i]["out"]).astype(np.float32)
            for i in range(NCORES)
        ],
        axis=0,
    )
    return out, res


def kernel(**inputs):
    out, _ = _run(inputs["x"], inputs["W"], inputs["b"])
    return out


def run_profiled(**inputs):
    out, res = _run(inputs["x"], inputs["W"], inputs["b"], trace=True)
    return out, res
